# revision 4
# baseline (speedup 1.0000x reference)
"""Trainium2 Bass kernel for DiagonalColCausalLinear.

Computes out[b,e,t] = sum_{s<t} x[b,e,s] * v[s] * d^(t-s) + x[b,e,t] * v2[t] + bias[t]
with d = clip(decay_value[1,0], 0.9, 1.0), v = weight, v2 = diag_weight.

Algorithm (per core; batch b sharded across 8 cores):
  Chunked causal scan along the sequence axis (chunk C=128) instead of the
  O(S^2) dense matmul -- O(B*E*S*C) work:
    - PE-transpose x into s-partition layout (exact, fp32)
    - within-chunk triangular matmuls vs precomputed T''_c (diag = v2)
    - cross-chunk carries via accumulating matmuls vs R (carry[e,c'] =
      contribution of all chunks < c' evaluated at chunk start)
    - carry applied as a per-partition bias during the PSUM->SBUF copy-out
      (ScalarE activation bias / DVE tensor_scalar, alternating), or via a
      rank-1 PE matmul against d^{t_l} when d != 1.
  All matmuls run in fp32 (full precision; 4 cycles/row on the PE).
"""
import numpy as np

import concourse.bass as bass
import concourse.mybir as mybir
import concourse.tile as tile
import concourse.bacc as bacc
from concourse import bass_utils

F32 = mybir.dt.float32

B, E, S = 8, 2048, 2048
N_CORES = 8
PT = 128            # partition tile
C = 128             # scan chunk == one k-subchunk
NCH = S // C        # 16 chunks
NE = E // PT        # 16 e-tiles per core
NSC = S // PT       # 16 s-subchunks

_prog_cache: dict = {}


def _build_constants(v: np.ndarray, v2: np.ndarray, d: float):
    """Host-side (tiny, O(S*C)) constant matrices encoding the decay structure."""
    Tm = np.zeros((NSC, PT, C), np.float32)
    Rm = np.zeros((NSC, PT, NCH), np.float32)
    t_local = np.arange(C)
    cc = np.arange(NCH)
    for sc in range(NSC):
        s_in_chunk = np.arange(PT)
        s_glob = sc * C + s_in_chunk
        diff = t_local[None, :] - s_in_chunk[:, None]
        with np.errstate(over="ignore", invalid="ignore"):
            Tm[sc] = np.where(diff > 0, v[s_glob][:, None] * (d ** np.maximum(diff, 0)), 0.0)
        Tm[sc][np.arange(PT), s_in_chunk] = v2[s_glob]
        # R[a, c'] = v[s] * d^(c'*C - s) for chunks c' > sc (carry to chunk start)
        expo = cc[None, :] * C - s_glob[:, None]
        with np.errstate(over="ignore", invalid="ignore"):
            Rm[sc] = np.where(cc[None, :] > sc, v[s_glob][:, None] * (d ** np.maximum(expo, 0)), 0.0)
    dpow = (d ** t_local).astype(np.float32).reshape(1, C)
    return Tm, Rm, dpow


def _build_program(d_is_one: bool, repeats: int = 1):
    key = (d_is_one, repeats)
    if key in _prog_cache:
        return _prog_cache[key]

    nc = bacc.Bacc("TRN2", target_bir_lowering=False, debug=False, num_devices=1)
    x_d = nc.dram_tensor("x", [E, S], F32, kind="ExternalInput").ap()
    tmat_d = nc.dram_tensor("tmat", [PT, NSC * C], F32, kind="ExternalInput").ap()
    rmat_d = nc.dram_tensor("rmat", [PT, NSC * NCH], F32, kind="ExternalInput").ap()
    ident_d = nc.dram_tensor("ident", [PT, PT], F32, kind="ExternalInput").ap()
    dpow_d = nc.dram_tensor("dpow", [1, C], F32, kind="ExternalInput").ap()
    out_d = nc.dram_tensor("out", [E, S], F32, kind="ExternalOutput").ap()

    with tile.TileContext(nc) as tc:
        with (
            tc.tile_pool(name="const", bufs=1) as cpool,
            tc.tile_pool(name="xin", bufs=3) as xpool,
            tc.tile_pool(name="xt", bufs=2) as xtpool,
            tc.tile_pool(name="outp", bufs=3) as opool,
            tc.tile_pool(name="small", bufs=3) as spool,
            tc.tile_pool(name="pst", bufs=2, space="PSUM") as pst,
            tc.tile_pool(name="psm", bufs=4, space="PSUM") as psm,
            tc.tile_pool(name="pscy", bufs=2, space="PSUM") as pscy,
        ):
            # constants (resident)
            tmat = cpool.tile([PT, NSC * C], F32, tag="tmat")
            nc.sync.dma_start(tmat[:, :], tmat_d[:, :])
            rmat = cpool.tile([PT, NSC * NCH], F32, tag="rmat")
            nc.sync.dma_start(rmat[:, :], rmat_d[:, :])
            ident = cpool.tile([PT, PT], F32, tag="ident")
            nc.sync.dma_start(ident[:, :], ident_d[:, :])
            dpow = cpool.tile([1, C], F32, tag="dpow")
            nc.sync.dma_start(dpow[:, :], dpow_d[:, :])

            for _rep in range(repeats):
                for i in range(NE):
                    x_sb = xpool.tile([PT, S], F32, tag="x")
                    nc.sync.dma_start(x_sb[:, :], x_d[i * PT:(i + 1) * PT, :])

                    # ---- forward transposes: x (e,s) -> xT (s,e), fp32 exact
                    xt_sb = xtpool.tile([PT, NSC * PT], F32, tag="xt")
                    for q in range(NSC // 4):           # 4 transposes per PSUM bank
                        ps_t = pst.tile([PT, 4 * PT], F32, tag="pt")
                        for w in range(4):
                            sc = q * 4 + w
                            nc.tensor.transpose(
                                ps_t[:, w * PT:(w + 1) * PT],
                                x_sb[:, sc * PT:(sc + 1) * PT],
                                ident[:, :],
                            )
                        nc.scalar.copy(xt_sb[:, q * 4 * PT:(q + 1) * 4 * PT], ps_t[:, :])

                    # ---- carry matmuls: psum_cy[e, c'] = sum_{s < c'*C} x[e,s]*v[s]*d^...
                    ps_cy = pscy.tile([PT, NCH], F32, tag="cy")
                    for sc in range(NSC):
                        nc.tensor.matmul(
                            ps_cy[:, :],
                            xt_sb[:, sc * PT:(sc + 1) * PT],
                            rmat[:, sc * NCH:(sc + 1) * NCH],
                            start=(sc == 0), stop=(sc == NSC - 1),
                        )

                    # ---- main within-chunk matmuls (one per chunk)
                    ps_m = [psm.tile([PT, 4 * C], F32, tag="m", name=f"ps_m{q}")
                            for q in range(NCH // 4)]
                    for c in range(NCH):
                        dst = ps_m[c // 4][:, (c % 4) * C:(c % 4 + 1) * C]
                        nc.tensor.matmul(
                            dst,
                            xt_sb[:, c * PT:(c + 1) * PT],
                            tmat[:, c * C:(c + 1) * C],
                            start=True, stop=d_is_one,
                        )

                    out_sb = opool.tile([PT, S], F32, tag="o")
                    cy_sb = spool.tile([PT, NCH], F32, tag="cys")
                    nc.scalar.copy(cy_sb[:, :], ps_cy[:, :])

                    if d_is_one:
                        # carry applied as per-partition bias during copy-out
                        for c in range(NCH):
                            src = ps_m[c // 4][:, (c % 4) * C:(c % 4 + 1) * C]
                            dstc = out_sb[:, c * C:(c + 1) * C]
                            if c % 2 == 0:
                                nc.scalar.add(dstc, src, cy_sb[:, c:c + 1])
                            else:
                                nc.vector.tensor_scalar_add(dstc, src, cy_sb[:, c:c + 1])
                    else:
                        # carry * d^{t_l} via rank-1 matmul into the main psum
                        ps_cyT = pscy.tile([NCH, PT], F32, tag="cyT")
                        nc.tensor.transpose(ps_cyT[:, :], cy_sb[:, :], ident[:, :])
                        cyT_sb = spool.tile([NCH, PT], F32, tag="cyTs")
                        nc.scalar.copy(cyT_sb[:, :], ps_cyT[:, :])
                        for c in range(NCH):
                            dst = ps_m[c // 4][:, (c % 4) * C:(c % 4 + 1) * C]
                            nc.tensor.matmul(
                                dst,
                                cyT_sb[c:c + 1, :],
                                dpow[:, :],
                                start=False, stop=True,
                            )
                        for c in range(NCH):
                            src = ps_m[c // 4][:, (c % 4) * C:(c % 4 + 1) * C]
                            dstc = out_sb[:, c * C:(c + 1) * C]
                            if c % 2 == 0:
                                nc.scalar.copy(dstc, src)
                            else:
                                nc.vector.tensor_copy(dstc, src)

                    nc.sync.dma_start(out_d[i * PT:(i + 1) * PT, :], out_sb[:, :])

    nc.compile()
    _prog_cache[key] = nc
    return nc


def _make_in_maps(x, Tm, Rm, dpow, ident):
    in_maps = []
    for b in range(N_CORES):
        in_maps.append({
            "x": x[b],
            "tmat": Tm.transpose(1, 0, 2).reshape(PT, NSC * C),
            "rmat": Rm.transpose(1, 0, 2).reshape(PT, NSC * NCH),
            "ident": ident,
            "dpow": dpow,
        })
    return in_maps


def kernel(x, weight, diag_weight, bias, decay_value):
    x = np.ascontiguousarray(np.asarray(x, dtype=np.float32))
    v = np.asarray(weight, dtype=np.float32).reshape(-1)
    v2 = np.asarray(diag_weight, dtype=np.float32).reshape(-1)
    bias = np.asarray(bias, dtype=np.float32).reshape(-1)
    d = float(np.clip(np.asarray(decay_value, dtype=np.float32)[1, 0], 0.9, 1.0))

    Tm, Rm, dpow = _build_constants(v, v2, d)
    ident = np.eye(PT, dtype=np.float32)
    nc = _build_program(d_is_one=(d == 1.0))

    in_maps = _make_in_maps(x, Tm, Rm, dpow, ident)
    res = bass_utils.run_bass_kernel_spmd(nc, in_maps, core_ids=list(range(N_CORES)))
    out = np.stack([res.results[b]["out"] for b in range(N_CORES)], axis=0)
    if np.any(bias):
        out = out + bias[None, None, :]
    return out
